# revision 14
# baseline (speedup 1.0000x reference)
"""Trainium2 Bass kernel for AnchorPostProcess (decode + top-k + NMS).

Self-contained: hardcodes shapes/constants for
B=8, A=6, C=16, H=W=160, STRIDE=8, PRE_K=2000, POST_K=100.

Strategy (verified on host against the reference):
  * score ranking uses max-logit m (sigmoid is monotone); sigmoid applied only
    to the <=100 outputs.
  * greedy NMS output (top-100 kept) only needs the global top-128 raw-score
    candidates; validity-masked candidates simply vacate ranks.
  * per-core (1 image/core): DMA the 16 logit planes, 15-op max tree -> m,
    per-partition top-8 pool (vector.max/max_index), kth_largest threshold ->
    exact global top-128, sparse_gather compaction -> (m,n) rows, matmul-based
    rank sort, per-candidate indirect-DMA gathers of deltas/logits, bit-exact
    anchor recompute, IoU matrix + Jacobi fixpoint NMS on TensorE, one-hot
    matmul output compaction.
"""

import numpy as np

P = 128
A = 6
C = 16
HW = 25600
NCH = 120
FW = 1200           # free width per partition (6 anchor types x 200)
F = 200
NSEL = 128
JITERS = 3
IMG = 1280.0
DW_CLAMP = 4.0
MIN_WH = 2.0
THRM = float(np.float32(np.log(np.float32(0.05) / np.float32(0.95))))  # -2.944439

_SIZES = np.array([32.0, 64.0, 128.0], np.float32)
_RATIOS = np.array([0.5, 2.0], np.float32)
_WS = (_SIZES[:, None] * np.sqrt(_RATIOS, dtype=np.float32)[None, :]).reshape(-1).astype(np.float32)
_HS = (_SIZES[:, None] / np.sqrt(_RATIOS, dtype=np.float32)[None, :]).reshape(-1).astype(np.float32)
CONSTS = np.zeros((1, 16), np.float32)
CONSTS[0, 0:6] = _WS
CONSTS[0, 6:12] = _HS
CONSTS_P = (40.0 * (np.arange(128) % 4)).astype(np.float32).reshape(128, 1)

# kth_largest quantile: k_adj = floor((1-q)*1023) must be 127 -> threshold =
# desc[128] (0-indexed), i.e. the 129th largest of the 1024 pool values.
KTH_Q = 1.0 - 127.5 / 1023.0


def build_nc():
    import concourse.bass as bass
    import concourse.mybir as mybir
    import concourse.tile as tile
    from concourse import bacc
    from concourse.masks import make_identity

    dt = mybir.dt
    Alu = mybir.AluOpType
    Act = mybir.ActivationFunctionType
    f32 = dt.float32

    nc = bacc.Bacc(None, target_bir_lowering=False)
    ho = nc.dram_tensor("head_out", [NCH, HW], f32, kind="ExternalInput")
    consts = nc.dram_tensor("consts", [1, 16], f32, kind="ExternalInput")
    consts_p = nc.dram_tensor("consts_p", [P, 1], f32, kind="ExternalInput")
    out_d = nc.dram_tensor("out", [P, 6], f32, kind="ExternalOutput")

    ho_flat = ho[:].rearrange("c (q e) -> (c q) e", e=1)

    with tile.TileContext(nc) as tc:
        with tc.tile_pool(name="sb", bufs=1) as sb, \
             tc.tile_pool(name="ps", bufs=1, space="PSUM") as ps:

            # ---------------- constants / statics ----------------
            idn = sb.tile([P, P], f32, tag="idn")
            make_identity(nc, idn[:])

            cst = sb.tile([1, 16], f32, tag="cst")
            nc.sync.dma_start(out=cst[:], in_=consts[:])
            cp = sb.tile([P, 1], f32, tag="cp")
            nc.sync.dma_start(out=cp[:], in_=consts_p[:])

            # iota matrix: every partition holds row 0..127
            iotaD_i = sb.tile([P, P], dt.int32, tag="iotaD_i")
            nc.gpsimd.iota(iotaD_i[:], pattern=[[1, P]], base=0, channel_multiplier=0)
            iotaD_M = sb.tile([P, P], f32, tag="iotaD_M")
            nc.vector.tensor_copy(iotaD_M[:], iotaD_i[:])
            iotaD_b = iotaD_M[:, :]

            pio_i = sb.tile([P, 1], dt.int32, tag="pio_i")
            nc.gpsimd.iota(pio_i[:], pattern=[[1, 1]], base=0, channel_multiplier=1)
            pio = sb.tile([P, 1], f32, tag="pio")
            nc.vector.tensor_copy(pio[:], pio_i[:])
            p200 = sb.tile([P, 1], f32, tag="p200")
            nc.vector.tensor_scalar_mul(p200[:], pio[:], 200.0)

            ioA_i = sb.tile([P, 6], dt.int32, tag="ioA_i")
            nc.gpsimd.iota(ioA_i[:], pattern=[[1, 6]], base=0, channel_multiplier=0)
            ioA = sb.tile([P, 6], f32, tag="ioA")
            nc.vector.tensor_copy(ioA[:], ioA_i[:])

            ioK_i = sb.tile([P, 4], dt.int32, tag="ioK_i")
            nc.gpsimd.iota(ioK_i[:], pattern=[[HW, 4]], base=0, channel_multiplier=0)
            ioK = sb.tile([P, 4], f32, tag="ioK")
            nc.vector.tensor_copy(ioK[:], ioK_i[:])

            ioL_i = sb.tile([P, C], dt.int32, tag="ioL_i")
            nc.gpsimd.iota(ioL_i[:], pattern=[[HW, C]], base=0, channel_multiplier=0)
            ioL = sb.tile([P, C], f32, tag="ioL")
            nc.vector.tensor_copy(ioL[:], ioL_i[:])

            ioC_i = sb.tile([P, C], dt.int32, tag="ioC_i")
            nc.gpsimd.iota(ioC_i[:], pattern=[[1, C]], base=0, channel_multiplier=0)
            ioC = sb.tile([P, C], f32, tag="ioC")
            nc.vector.tensor_copy(ioC[:], ioC_i[:])

            # iota over sparse-gather order (seq = p + 16*f) on 16 partitions
            isg_i = sb.tile([16, 8], dt.int32, tag="isg_i")
            nc.gpsimd.iota(isg_i[:], pattern=[[16, 8]], base=0, channel_multiplier=1)
            isg = sb.tile([16, 8], f32, tag="isg")
            nc.vector.tensor_copy(isg[:], isg_i[:])

            negc = sb.tile([16, 8], f32, tag="negc")
            nc.gpsimd.memset(negc[:], -1.0e25)

            z1 = sb.tile([P, 1], f32, tag="z1")
            nc.gpsimd.memset(z1[:], 0.0)
            id1 = sb.tile([1, 1], f32, tag="id1")
            nc.gpsimd.memset(id1[:], 1.0)
            z1r = sb.tile([1, P], f32, tag="z1r")
            nc.gpsimd.memset(z1r[:], 0.0)

            # preload exp table early (overlaps DMA)
            dum = sb.tile([P, 1], f32, tag="dum")
            nc.gpsimd.memset(dum[:], 0.0)
            nc.scalar.activation(dum[:], dum[:], Act.Exp)

            # ---------------- phase 1: logits DMA + max tree ----------------
            Ls = []
            for c in range(C):
                Lc = sb.tile([P, FW], f32, tag=f"L{c}")
                src = ho[:].rearrange("(a k) q -> a k q", k=20)[:, 4 + c, :]
                src = src.rearrange("a (p f) -> p a f", p=P)
                nc.sync.dma_start(out=Lc[:].rearrange("p (a f) -> p a f", a=A), in_=src)
                Ls.append(Lc)

            m = sb.tile([P, FW], f32, tag="m")
            nc.vector.tensor_tensor(m[:], Ls[0][:], Ls[1][:], Alu.max)
            for c in range(2, C):
                nc.vector.tensor_tensor(m[:], m[:], Ls[c][:], Alu.max)

            # ---------------- phase 2: pool (per-partition top-8) ----------------
            m8 = sb.tile([P, 8], f32, tag="m8")
            nc.vector.max(m8[:], m[:])
            i8 = sb.tile([P, 8], dt.uint16, tag="i8")
            nc.vector.max_index(i8[:], m8[:], m[:])
            i8f = sb.tile([P, 8], f32, tag="i8f")
            nc.vector.tensor_copy(i8f[:], i8[:])

            # n8 = 25400*a + 200*p + ffull ; a = sum_t [ffull >= 200t]
            a8 = sb.tile([P, 8], f32, tag="a8")
            z8 = sb.tile([P, 8], f32, tag="z8")
            nc.gpsimd.memset(z8[:], 0.0)
            prev = z8
            for t in range(1, 6):
                cur = sb.tile([P, 8], f32, tag=f"a8c{t}")
                nc.vector.scalar_tensor_tensor(cur[:], i8f[:], 200.0 * t, prev[:],
                                               Alu.is_ge, Alu.add)
                prev = cur
            a8 = prev
            n8 = sb.tile([P, 8], f32, tag="n8")
            nc.vector.scalar_tensor_tensor(n8[:], a8[:], 25400.0, i8f[:], Alu.mult, Alu.add)
            nc.vector.tensor_scalar(n8[:], n8[:], p200[:, 0:1], None, Alu.add)

            # ---------------- phase 3: threshold + selection ----------------
            kth = sb.tile([1, 2], f32, tag="kth")
            nc.gpsimd.kth_largest(kth[:], m8[:], n_per_lane=8, k=NSEL, quantile=KTH_Q)
            tb = sb.tile([P, 1], f32, tag="tb")
            nc.gpsimd.partition_broadcast(tb[:], kth[0:1, 1:2])
            sel8 = sb.tile([P, 8], f32, tag="sel8")
            nc.vector.tensor_scalar(sel8[:], m8[:], tb[:, 0:1], None, Alu.is_gt)

            # enc = sel*val + (sel-1)
            s1 = sb.tile([P, 8], f32, tag="s1")
            nc.vector.tensor_scalar(s1[:], sel8[:], 1.0, None, Alu.subtract)
            encm = sb.tile([P, 8], f32, tag="encm")
            nc.vector.tensor_tensor(encm[:], sel8[:], m8[:], Alu.mult)
            nc.vector.tensor_tensor(encm[:], encm[:], s1[:], Alu.add)
            encn = sb.tile([P, 8], f32, tag="encn")
            nc.vector.tensor_tensor(encn[:], sel8[:], n8[:], Alu.mult)
            nc.vector.tensor_tensor(encn[:], encn[:], s1[:], Alu.add)

            # transpose [128,8] -> [8,128] -> reshape [16,64] -> sparse_gather
            rows = {}
            for nm, enc in (("m", encm), ("n", encn)):
                pt = ps.tile([8, P], f32, tag=f"pt_{nm}")
                nc.tensor.transpose(pt[:], enc[:], idn[:])
                st = sb.tile([8, P], f32, tag=f"st_{nm}")
                nc.vector.tensor_copy(st[:], pt[:])
                sg_in = sb.tile([16, 64], f32, tag=f"sgin_{nm}")
                nc.sync.dma_start(out=sg_in[:], in_=st[:])
                sgo = sb.tile([16, 8], f32, tag=f"sgo_{nm}")
                nf = sb.tile([1, 1], dt.uint32, tag=f"nf_{nm}")
                nc.gpsimd.sparse_gather(sgo[:], sg_in[:], num_found=nf[:])
                rows[nm] = (sgo, nf)

            # tail masking for the m-plane (if num_found < 128)
            sgo_m, nf_m = rows["m"]
            nf_f = sb.tile([1, 1], f32, tag="nf_f")
            nc.vector.tensor_copy(nf_f[:], nf_m[:])
            nfb = sb.tile([16, 1], f32, tag="nfb")
            nc.gpsimd.partition_broadcast(nfb[:], nf_f[0:1, 0:1])
            mskT = sb.tile([16, 8], dt.uint8, tag="mskT")
            nc.vector.tensor_scalar(mskT[:], isg[:], nfb[:, 0:1], None, Alu.is_ge)
            nc.vector.copy_predicated(sgo_m[:], mskT[:], negc[:])

            # rows [1,128]
            m_row = sb.tile([1, P], f32, tag="m_row")
            nc.sync.dma_start(out=m_row[:], in_=sgo_m[:])
            n_row = sb.tile([1, P], f32, tag="n_row")
            nc.sync.dma_start(out=n_row[:], in_=rows["n"][0][:])

            m_rowM = sb.tile([P, P], f32, tag="m_rowM")
            nc.gpsimd.partition_broadcast(m_rowM[:], m_row[:])
            n_rowM = sb.tile([P, P], f32, tag="n_rowM")
            nc.gpsimd.partition_broadcast(n_rowM[:], n_row[:])
            m_rowB = m_rowM[:, :]
            n_rowB = n_rowM[:, :]

            # ---------------- phase 4: rank + one-hot sort ----------------
            rank8 = sb.tile([P, 8], f32, tag="rank8")
            for r in range(8):
                e_r = sb.tile([P, P], f32, tag="e_r")
                nc.vector.tensor_scalar(e_r[:], m_rowB, m8[:, r:r + 1], None, Alu.is_equal)
                l_r = sb.tile([P, P], f32, tag="l_r")
                nc.vector.tensor_scalar(l_r[:], n_rowB, n8[:, r:r + 1], None, Alu.is_lt)
                u_r = sb.tile([P, P], f32, tag="u_r")
                nc.vector.tensor_tensor(u_r[:], e_r[:], l_r[:], Alu.mult)
                w_r = sb.tile([P, P], f32, tag="w_r")
                nc.vector.scalar_tensor_tensor(w_r[:], m_rowB, m8[:, r:r + 1], u_r[:],
                                               Alu.is_gt, Alu.add,
                                               accum_out=rank8[:, r:r + 1])

            # mask unselected slots out of the rank space
            p9 = sb.tile([P, 8], f32, tag="p9")
            nc.vector.tensor_scalar(p9[:], sel8[:], 0.0, 999.0, Alu.is_equal, Alu.mult)
            rankm = sb.tile([P, 8], f32, tag="rankm")
            nc.vector.tensor_tensor(rankm[:], rank8[:], p9[:], Alu.add)

            # V[p, r, d]: d = (m, ffull, p, pm40)
            V = sb.tile([P, 8, 4], f32, tag="V")
            nc.vector.tensor_copy(V[:, :, 0], m8[:])
            nc.vector.tensor_copy(V[:, :, 1], i8f[:])
            nc.vector.tensor_copy(V[:, :, 2], pio[:, 0:1].to_broadcast([P, 8]))
            nc.vector.tensor_copy(V[:, :, 3], cp[:, 0:1].to_broadcast([P, 8]))

            psV = ps.tile([P, 4], f32, tag="psV")
            for r in range(8):
                PT_r = sb.tile([P, P], f32, tag="PT_r")
                nc.vector.tensor_scalar(PT_r[:], iotaD_b, rankm[:, r:r + 1], None,
                                        Alu.is_equal)
                nc.tensor.matmul(psV[:], PT_r[:], V[:, r, :], start=(r == 0),
                                 stop=(r == 7))
            m_s = sb.tile([P, 1], f32, tag="m_s")
            nc.vector.tensor_copy(m_s[:], psV[:, 0:1])
            ff_s = sb.tile([P, 1], f32, tag="ff_s")
            nc.vector.tensor_copy(ff_s[:], psV[:, 1:2])
            p_s = sb.tile([P, 1], f32, tag="p_s")
            nc.vector.tensor_copy(p_s[:], psV[:, 2:3])
            pm40_s = sb.tile([P, 1], f32, tag="pm40_s")
            nc.vector.tensor_copy(pm40_s[:], psV[:, 3:4])

            # ---------------- phase 5: per-candidate decode ----------------
            # a = ffull // 200 (compare chain); f = ffull - 200a
            prev = z1
            for t in range(1, 6):
                cur = sb.tile([P, 1], f32, tag=f"as{t}")
                nc.vector.scalar_tensor_tensor(cur[:], ff_s[:], 200.0 * t, prev[:],
                                               Alu.is_ge, Alu.add)
                prev = cur
            a_s = prev
            f_s = sb.tile([P, 1], f32, tag="f_s")
            nc.vector.scalar_tensor_tensor(f_s[:], a_s[:], -200.0, ff_s[:],
                                           Alu.mult, Alu.add)
            pos = sb.tile([P, 1], f32, tag="pos")
            nc.vector.scalar_tensor_tensor(pos[:], p_s[:], 200.0, f_s[:],
                                           Alu.mult, Alu.add)
            # w = (pm40 + f) mod 160 via single range reduction; h = (pos-w)/160
            vv = sb.tile([P, 1], f32, tag="vv")
            nc.vector.tensor_add(vv[:], pm40_s[:], f_s[:])
            sge = sb.tile([P, 1], f32, tag="sge")
            nc.vector.tensor_scalar(sge[:], vv[:], 160.0, None, Alu.is_ge)
            wq = sb.tile([P, 1], f32, tag="wq")
            nc.vector.scalar_tensor_tensor(wq[:], sge[:], -160.0, vv[:],
                                           Alu.mult, Alu.add)
            hq = sb.tile([P, 1], f32, tag="hq")
            nc.vector.scalar_tensor_tensor(hq[:], wq[:], -1.0, pos[:], Alu.mult, Alu.add)
            nc.vector.tensor_scalar(hq[:], hq[:], 1.0 / 160.0, None, Alu.mult)

            cxg = sb.tile([P, 1], f32, tag="cxg")
            nc.vector.tensor_scalar(cxg[:], wq[:], 0.5, 8.0, Alu.add, Alu.mult)
            cyg = sb.tile([P, 1], f32, tag="cyg")
            nc.vector.tensor_scalar(cyg[:], hq[:], 0.5, 8.0, Alu.add, Alu.mult)

            # anchor ws/hs select by a
            ohA = sb.tile([P, 6], f32, tag="ohA")
            nc.vector.tensor_scalar(ohA[:], ioA[:], a_s[:, 0:1], None, Alu.is_equal)
            cstM = sb.tile([P, 16], f32, tag="cstM")
            nc.gpsimd.partition_broadcast(cstM[:], cst[:])
            ws_s = sb.tile([P, 1], f32, tag="ws_s")
            dmy = sb.tile([P, 6], f32, tag="dmy")
            nc.vector.scalar_tensor_tensor(dmy[:], ohA[:], 1.0, cstM[:, 0:6],
                                           Alu.mult, Alu.mult, accum_out=ws_s[:])
            hs_s = sb.tile([P, 1], f32, tag="hs_s")
            dmy2 = sb.tile([P, 6], f32, tag="dmy2")
            nc.vector.scalar_tensor_tensor(dmy2[:], ohA[:], 1.0, cstM[:, 6:12],
                                           Alu.mult, Alu.mult, accum_out=hs_s[:])

            # anc values (bit-exact reference replication)
            wsh = sb.tile([P, 1], f32, tag="wsh")
            nc.vector.tensor_scalar(wsh[:], ws_s[:], 0.5, None, Alu.mult)
            hsh = sb.tile([P, 1], f32, tag="hsh")
            nc.vector.tensor_scalar(hsh[:], hs_s[:], 0.5, None, Alu.mult)
            anc0 = sb.tile([P, 1], f32, tag="anc0")
            nc.vector.tensor_sub(anc0[:], cxg[:], wsh[:])
            anc2 = sb.tile([P, 1], f32, tag="anc2")
            nc.vector.tensor_add(anc2[:], cxg[:], wsh[:])
            anc1 = sb.tile([P, 1], f32, tag="anc1")
            nc.vector.tensor_sub(anc1[:], cyg[:], hsh[:])
            anc3 = sb.tile([P, 1], f32, tag="anc3")
            nc.vector.tensor_add(anc3[:], cyg[:], hsh[:])
            aw = sb.tile([P, 1], f32, tag="aw")
            nc.vector.tensor_sub(aw[:], anc2[:], anc0[:])
            ah = sb.tile([P, 1], f32, tag="ah")
            nc.vector.tensor_sub(ah[:], anc3[:], anc1[:])
            acx = sb.tile([P, 1], f32, tag="acx")
            nc.vector.scalar_tensor_tensor(acx[:], aw[:], 0.5, anc0[:], Alu.mult, Alu.add)
            acy = sb.tile([P, 1], f32, tag="acy")
            nc.vector.scalar_tensor_tensor(acy[:], ah[:], 0.5, anc1[:], Alu.mult, Alu.add)

            # indirect gathers: deltas [128,4], logits [128,16]
            base = sb.tile([P, 1], f32, tag="base")
            nc.vector.scalar_tensor_tensor(base[:], a_s[:], 512000.0, pos[:],
                                           Alu.mult, Alu.add)
            # offsets for all 20 channels of this candidate (stride HW apart)
            ioKL_i = sb.tile([P, 20], dt.int32, tag="ioKL_i")
            nc.gpsimd.iota(ioKL_i[:], pattern=[[HW, 20]], base=0, channel_multiplier=0)
            ioKL = sb.tile([P, 20], f32, tag="ioKL")
            nc.vector.tensor_copy(ioKL[:], ioKL_i[:])
            offA = sb.tile([P, 20], f32, tag="offA")
            nc.vector.tensor_scalar(offA[:], ioKL[:], base[:, 0:1], None, Alu.add)
            offAi = sb.tile([P, 20], dt.int32, tag="offAi")
            nc.vector.tensor_copy(offAi[:], offA[:])

            # HW indirect DMA: one offset per partition, contiguous run of
            # dest-free-size -> 20 separate [128,1] gathers
            ga = sb.tile([P, 20], f32, tag="ga")
            for k in range(20):
                nc.gpsimd.indirect_dma_start(
                    out=ga[:, k:k + 1], out_offset=None, in_=ho_flat,
                    in_offset=bass.IndirectOffsetOnAxis(ap=offAi[:, k:k + 1], axis=0))
            dg = ga[:, 0:4]
            lg = ga[:, 4:20]

            # decode
            dwc = sb.tile([P, 1], f32, tag="dwc")
            nc.vector.tensor_scalar(dwc[:], ga[:, 2:3], DW_CLAMP, None, Alu.min)
            dhc = sb.tile([P, 1], f32, tag="dhc")
            nc.vector.tensor_scalar(dhc[:], ga[:, 3:4], DW_CLAMP, None, Alu.min)
            ew = sb.tile([P, 1], f32, tag="ew")
            nc.scalar.activation(ew[:], dwc[:], Act.Exp)
            eh = sb.tile([P, 1], f32, tag="eh")
            nc.scalar.activation(eh[:], dhc[:], Act.Exp)
            gw = sb.tile([P, 1], f32, tag="gw")
            nc.vector.tensor_mul(gw[:], aw[:], ew[:])
            gh = sb.tile([P, 1], f32, tag="gh")
            nc.vector.tensor_mul(gh[:], ah[:], eh[:])
            ux = sb.tile([P, 1], f32, tag="ux")
            nc.vector.tensor_mul(ux[:], ga[:, 0:1], aw[:])
            gcx = sb.tile([P, 1], f32, tag="gcx")
            nc.vector.tensor_add(gcx[:], acx[:], ux[:])
            uy = sb.tile([P, 1], f32, tag="uy")
            nc.vector.tensor_mul(uy[:], ga[:, 1:2], ah[:])
            gcy = sb.tile([P, 1], f32, tag="gcy")
            nc.vector.tensor_add(gcy[:], acy[:], uy[:])
            gwh = sb.tile([P, 1], f32, tag="gwh")
            nc.vector.tensor_scalar(gwh[:], gw[:], 0.5, None, Alu.mult)
            ghh = sb.tile([P, 1], f32, tag="ghh")
            nc.vector.tensor_scalar(ghh[:], gh[:], 0.5, None, Alu.mult)

            x1c = sb.tile([P, 1], f32, tag="x1c")
            nc.vector.tensor_sub(x1c[:], gcx[:], gwh[:])
            nc.vector.tensor_scalar(x1c[:], x1c[:], 0.0, IMG, Alu.max, Alu.min)
            x2c = sb.tile([P, 1], f32, tag="x2c")
            nc.vector.tensor_add(x2c[:], gcx[:], gwh[:])
            nc.vector.tensor_scalar(x2c[:], x2c[:], 0.0, IMG, Alu.max, Alu.min)
            y1c = sb.tile([P, 1], f32, tag="y1c")
            nc.vector.tensor_sub(y1c[:], gcy[:], ghh[:])
            nc.vector.tensor_scalar(y1c[:], y1c[:], 0.0, IMG, Alu.max, Alu.min)
            y2c = sb.tile([P, 1], f32, tag="y2c")
            nc.vector.tensor_add(y2c[:], gcy[:], ghh[:])
            nc.vector.tensor_scalar(y2c[:], y2c[:], 0.0, IMG, Alu.max, Alu.min)

            # validity
            wv = sb.tile([P, 1], f32, tag="wv")
            nc.vector.tensor_sub(wv[:], x2c[:], x1c[:])
            hv = sb.tile([P, 1], f32, tag="hv")
            nc.vector.tensor_sub(hv[:], y2c[:], y1c[:])
            q1 = sb.tile([P, 1], f32, tag="q1")
            nc.vector.tensor_scalar(q1[:], wv[:], MIN_WH, None, Alu.subtract)
            q2 = sb.tile([P, 1], f32, tag="q2")
            nc.vector.tensor_scalar(q2[:], hv[:], MIN_WH, None, Alu.subtract)
            qq = sb.tile([P, 1], f32, tag="qq")
            nc.vector.tensor_tensor(qq[:], q1[:], q2[:], Alu.min)
            q3 = sb.tile([P, 1], f32, tag="q3")
            nc.vector.tensor_scalar(q3[:], m_s[:], THRM, None, Alu.subtract)
            nc.vector.tensor_tensor(qq[:], qq[:], q3[:], Alu.min)
            vmask = sb.tile([P, 1], f32, tag="vmask")
            nc.vector.tensor_scalar(vmask[:], qq[:], 0.0, None, Alu.is_ge)

            # sigmoid + cls
            sig = sb.tile([P, 1], f32, tag="sig")
            nc.scalar.activation(sig[:], m_s[:], Act.Sigmoid)
            eqc = sb.tile([P, C], f32, tag="eqc")
            nc.vector.tensor_scalar(eqc[:], ga[:, 4:20], m_s[:, 0:1], None, Alu.is_equal)
            cls_s = sb.tile([P, 1], f32, tag="cls_s")
            dmy3 = sb.tile([P, C], f32, tag="dmy3")
            nc.vector.scalar_tensor_tensor(dmy3[:], eqc[:], 1.0, ioC[:],
                                           Alu.mult, Alu.mult, accum_out=cls_s[:])

            # ---------------- phase 6: NMS ----------------
            bz = []
            for src_t in (x1c, y1c, x2c, y2c):
                z = sb.tile([P, 1], f32, tag=f"bz{len(bz)}")
                nc.vector.tensor_mul(z[:], src_t[:], vmask[:])
                bz.append(z)
            t1 = sb.tile([P, 1], f32, tag="t1")
            nc.vector.tensor_sub(t1[:], bz[2][:], bz[0][:])
            t2 = sb.tile([P, 1], f32, tag="t2")
            nc.vector.tensor_sub(t2[:], bz[3][:], bz[1][:])
            areaZ = sb.tile([P, 1], f32, tag="areaZ")
            nc.vector.tensor_mul(areaZ[:], t1[:], t2[:])

            Trow = sb.tile([P, 5], f32, tag="Trow")
            for k, src_t in enumerate(bz + [areaZ]):
                nc.vector.tensor_copy(Trow[:, k:k + 1], src_t[:])
            rows5p = ps.tile([5, P], f32, tag="rows5p")
            nc.tensor.transpose(rows5p[:], Trow[:], idn[:])
            rows5 = sb.tile([5, P], f32, tag="rows5")
            nc.vector.tensor_copy(rows5[:], rows5p[:])
            row540 = sb.tile([1, 5 * P], f32, tag="row540")
            nc.sync.dma_start(out=row540[:], in_=rows5[:])
            rows5M = sb.tile([P, 5 * P], f32, tag="rows5M")
            nc.gpsimd.partition_broadcast(rows5M[:], row540[:])

            def rowB(k):
                return rows5M[:, k * P:(k + 1) * P]

            def colB(t):
                return t[:, 0:1].to_broadcast([P, P])

            ltx = sb.tile([P, P], f32, tag="ltx")
            nc.vector.tensor_tensor(ltx[:], colB(bz[0]), rowB(0), Alu.max)
            rbx = sb.tile([P, P], f32, tag="rbx")
            nc.vector.tensor_tensor(rbx[:], colB(bz[2]), rowB(2), Alu.min)
            iw = sb.tile([P, P], f32, tag="iw")
            nc.vector.tensor_sub(iw[:], rbx[:], ltx[:])
            nc.vector.tensor_scalar(iw[:], iw[:], 0.0, None, Alu.max)
            lty = sb.tile([P, P], f32, tag="lty")
            nc.vector.tensor_tensor(lty[:], colB(bz[1]), rowB(1), Alu.max)
            rby = sb.tile([P, P], f32, tag="rby")
            nc.vector.tensor_tensor(rby[:], colB(bz[3]), rowB(3), Alu.min)
            ih = sb.tile([P, P], f32, tag="ih")
            nc.vector.tensor_sub(ih[:], rby[:], lty[:])
            nc.vector.tensor_scalar(ih[:], ih[:], 0.0, None, Alu.max)
            inter = sb.tile([P, P], f32, tag="inter")
            nc.vector.tensor_mul(inter[:], iw[:], ih[:])
            asum = sb.tile([P, P], f32, tag="asum")
            nc.vector.tensor_tensor(asum[:], colB(areaZ), rowB(4), Alu.add)
            sup = sb.tile([P, P], f32, tag="sup")
            nc.vector.scalar_tensor_tensor(sup[:], inter[:], 3.0, asum[:],
                                           Alu.mult, Alu.is_gt)
            Msup = sb.tile([P, P], f32, tag="Msup")
            nc.gpsimd.affine_select(out=Msup[:], in_=sup[:], pattern=[[1, P]],
                                    compare_op=Alu.is_gt, fill=0.0, base=0,
                                    channel_multiplier=-1)

            kv = sb.tile([P, 1], f32, tag="kv0")
            nc.gpsimd.memset(kv[:], 1.0)
            for it in range(JITERS):
                psS = ps.tile([P, 1], f32, tag="psS")
                nc.tensor.matmul(psS[:], Msup[:], kv[:], start=True, stop=True)
                kv = sb.tile([P, 1], f32, tag=f"kv{it + 1}")
                nc.vector.tensor_scalar(kv[:], psS[:], 0.0, None, Alu.is_equal)

            final = sb.tile([P, 1], f32, tag="final")
            nc.vector.tensor_mul(final[:], kv[:], vmask[:])

            # ---------------- phase 7: output compaction ----------------
            krp = ps.tile([1, P], f32, tag="krp")
            nc.tensor.transpose(krp[:], final[:], idn[:])
            k_row = sb.tile([1, P], f32, tag="k_row")
            nc.vector.tensor_copy(k_row[:], krp[:])
            incl = sb.tile([1, P], f32, tag="incl")
            nc.vector.tensor_tensor_scan(incl[:], k_row[:], z1r[:], 0.0,
                                         Alu.add, Alu.max)
            pos_r = sb.tile([1, P], f32, tag="pos_r")
            nc.vector.tensor_scalar(pos_r[:], incl[:], 1.0, None, Alu.subtract)
            qd = sb.tile([1, P], f32, tag="qd")
            nc.vector.tensor_scalar(qd[:], k_row[:], 0.0, 999.0, Alu.is_equal, Alu.mult)
            dest2 = sb.tile([1, P], f32, tag="dest2")
            nc.vector.tensor_mul(dest2[:], pos_r[:], k_row[:])
            nc.vector.tensor_add(dest2[:], dest2[:], qd[:])
            d2p = ps.tile([P, 1], f32, tag="d2p")
            nc.tensor.transpose(d2p[:], dest2[:], id1[:])
            d2c = sb.tile([P, 1], f32, tag="d2c")
            nc.vector.tensor_copy(d2c[:], d2p[:])

            FT = sb.tile([P, P], f32, tag="FT")
            nc.vector.tensor_scalar(FT[:], iotaD_b, d2c[:, 0:1], None, Alu.is_equal)

            V2 = sb.tile([P, 6], f32, tag="V2")
            for k, src_t in enumerate((x1c, y1c, x2c, y2c, sig, cls_s)):
                nc.vector.tensor_copy(V2[:, k:k + 1], src_t[:])
            outp = ps.tile([P, 6], f32, tag="outp")
            nc.tensor.matmul(outp[:], FT[:], V2[:], start=True, stop=True)
            outs = sb.tile([P, 6], f32, tag="outs")
            nc.vector.tensor_copy(outs[:], outp[:])
            nc.sync.dma_start(out=out_d[:], in_=outs[:])

    nc.compile()
    return nc


_NC = None


def _get_nc():
    global _NC
    if _NC is None:
        _NC = build_nc()
    return _NC


def kernel(anchors: np.ndarray, head_out: np.ndarray):
    from concourse.bass_utils import run_bass_kernel_spmd

    nc = _get_nc()
    B = head_out.shape[0]
    in_maps = [
        {"head_out": np.ascontiguousarray(head_out[b].reshape(NCH, HW), dtype=np.float32),
         "consts": CONSTS, "consts_p": CONSTS_P}
        for b in range(B)
    ]
    res = run_bass_kernel_spmd(nc, in_maps, core_ids=list(range(B)))
    outs = [res.results[b]["out"] for b in range(B)]
    boxes = np.stack([o[:100, 0:4] for o in outs]).astype(np.float32)
    scores = np.stack([o[:100, 4] for o in outs]).astype(np.float32)
    cls = np.stack([np.rint(o[:100, 5]) for o in outs]).astype(np.int32)
    return boxes, scores, cls
